# revision 14
# baseline (speedup 1.0000x reference)
"""Trainium2 Bass kernel for nn_MultiHeadAttention_79465484911033.

Sharding: 8 cores = 2 batches x 4 head-groups (4 heads each of 16).
Each core: QKV projection for its heads (column-parallel), RoPE
(spatial+temporal angles composed into one rotation), causal attention,
swish, and a row-parallel partial output projection. Host sums the 4
partials per batch and adds b_out.

v2 design notes (vs the 187.7us baseline):
- RoPE shuffle via DVE stream_shuffle (32-lane pair-swap permute) with
  the sign folded into the sin table; kills the PE shuffle matmuls and
  one psum drain per chunk.
- q/k bias added during the psum drain (ACT Identity+bias-vector for
  slab-phase chunks, DVE tensor_scalar for attention-phase chunks);
  kills the bias matmuls.
- Scores diagonal blocks compute only the causal q-range (N=512-128d).
- attn@v: both heads of a pair run as one col-tiled concurrent MM pair
  (M=64 at col-groups 0/64) on full [128,128] v stationary (FWL-able);
  the softmax denominators for all 4 heads accumulate via 4 concurrent
  col-tiled M=1 ones-matmuls into a separate psum bank. 3 psum-stream
  slots per k-block instead of 4.
- swish: o/(2S) with S reshaped [128,16] via DMA for the cheap DVE
  reciprocal, broadcast via gpsimd mid-kernel but via PE K=1 matmuls
  for the last slot (keeps PE warm through the tail, avoids the ~2us
  gpsimd wake+drain latency before the final oproj).
- attention phase is ACT(exp)-bound; PE filled with pair-1 projection
  chunks and oproj blocks spliced between j-steps.
- oproj drains on DVE mid-kernel, DVE+ACT alternating for the tail
  block which uses the (then-free) score psum banks.
"""

import sys

for _p in ("/opt/trn_rl_repo", "/root/.axon_site/_ro/trn_rl_repo"):
    if _p not in sys.path:
        sys.path.append(_p)

import numpy as np
import ml_dtypes

import concourse.bass as bass
import concourse.mybir as mybir
import concourse.tile as tile
from concourse import bacc
from concourse.bass_utils import run_bass_kernel_spmd

F32 = mybir.dt.float32
BF16 = mybir.dt.bfloat16
AF = mybir.ActivationFunctionType
ALU = mybir.AluOpType

B, L, H = 2, 2048, 1024
NH, HD = 16, 64
NT, LS, L1D = 8, 256, 16
N_CORES = 8
HPC = 4               # heads per core
NKC = H // 128        # 8 contraction chunks
NL = L // 128         # 16 L chunks of 128
NLQ = L // 512        # 4 L tiles of 512
LAG = 2

_CACHE = {}

PAIR_SWAP = []
for _i in range(16):
    PAIR_SWAP += [2 * _i + 1, 2 * _i]


def _build(debug_taps=False):
    nc = bacc.Bacc("TRN2", target_bir_lowering=False, debug=False,
                   enable_asserts=True, num_devices=N_CORES)
    taps = {}
    if debug_taps:
        taps["qkr"] = nc.dram_tensor("dbg_qkr", [4, 128, L], BF16,
                                     kind="ExternalOutput")
        taps["v"] = nc.dram_tensor("dbg_v", [NL, 128, 256], BF16,
                                   kind="ExternalOutput")
        taps["oT"] = nc.dram_tensor("dbg_oT", [2, 128, L], BF16,
                                    kind="ExternalOutput")
        taps["ssb"] = nc.dram_tensor("dbg_ssb", [NLQ, 97, 512], F32,
                                     kind="ExternalOutput")
        taps["rec1"] = nc.dram_tensor("dbg_rec1", [NLQ, 1, 2048], BF16,
                                      kind="ExternalOutput")
        taps["osb"] = nc.dram_tensor("dbg_osb", [NLQ, 2, 128, 512], BF16,
                                     kind="ExternalOutput")

    xt_d = nc.dram_tensor("xt", [H, L], BF16, kind="ExternalInput")
    wqk_d = nc.dram_tensor("wqk", [H, 512], BF16, kind="ExternalInput")
    bqk_d = nc.dram_tensor("bqkc", [128, 4], F32, kind="ExternalInput")
    wv_d = nc.dram_tensor("wv", [H, 256], BF16, kind="ExternalInput")
    bv_d = nc.dram_tensor("bv", [1, 256], BF16, kind="ExternalInput")
    cos_d = nc.dram_tensor("cosrep", [128, L], BF16, kind="ExternalInput")
    sin_d = nc.dram_tensor("sinsig", [128, L], BF16, kind="ExternalInput")
    wo_d = nc.dram_tensor("woT", [256, 1024], BF16, kind="ExternalInput")
    out_d = nc.dram_tensor("out", [L, H], BF16, kind="ExternalOutput")

    with tile.TileContext(nc) as tc:
        with (
            tc.tile_pool(name="const", bufs=1) as cpool,
            tc.tile_pool(name="xt", bufs=1) as xpool,
            tc.tile_pool(name="w", bufs=1) as wpool,
            tc.tile_pool(name="qk", bufs=1) as qkpool,
            tc.tile_pool(name="v", bufs=1) as vpool,
            tc.tile_pool(name="work", bufs=1) as work,
            tc.tile_pool(name="pt", bufs=10) as ptpool,
            tc.tile_pool(name="sw", bufs=2) as swpool,
            tc.tile_pool(name="ps", bufs=1, space="PSUM") as psum,
        ):
            cos_t = cpool.tile([128, L], BF16, tag="cos")
            sin_t = cpool.tile([128, L], BF16, tag="sin")
            bqk_t = cpool.tile([128, 4], F32, tag="bqk")
            bv_t = cpool.tile([1, 256], BF16, tag="bv")
            ones_t = cpool.tile([1, 512], BF16, tag="ones")
            onesc_t = cpool.tile([128, 1], BF16, tag="onesc")
            nc.vector.memset(ones_t[:], 1.0)
            nc.vector.memset(onesc_t[:], 1.0)

            # prime the exp/tanh activation table set during initial DMA
            prm = cpool.tile([1, 8], F32, tag="prm")
            prm2 = cpool.tile([1, 8], F32, tag="prm2")
            nc.vector.memset(prm[:], 0.0)
            nc.scalar.activation(prm2[:], prm[:], AF.Exp)

            wv_t = []
            for k in range(NKC):
                t = wpool.tile([128, 256], BF16, tag=f"wv{k}", name=f"wv{k}")
                nc.sync.dma_start(t[:], wv_d[k * 128:(k + 1) * 128, :])
                wv_t.append(t)
            nc.sync.dma_start(bv_t[:], bv_d[:])
            xt_t = [xpool.tile([128, L], BF16, tag=f"xt{k}", name=f"xt{k}")
                    for k in range(NKC)]
            wqk_t = [wpool.tile([128, 512], BF16, tag=f"wqk{k}",
                                name=f"wqk{k}") for k in range(NKC)]

            def dma_slab(s):
                sl = bass.ts(s, 512)
                for k in range(NKC):
                    nc.sync.dma_start(xt_t[k][:, sl],
                                      xt_d[k * 128:(k + 1) * 128, sl])
                nc.sync.dma_start(cos_t[:, sl], cos_d[:, sl])
                nc.sync.dma_start(sin_t[:, sl], sin_d[:, sl])

            dma_slab(0)
            for k in range(NKC):
                nc.sync.dma_start(wqk_t[k][:], wqk_d[k * 128:(k + 1) * 128, :])
            nc.sync.dma_start(bqk_t[:], bqk_d[:])
            for s in range(1, NLQ):
                dma_slab(s)
            wo_t = []
            for g in range(2):
                t = wpool.tile([128, 1024], BF16, tag=f"wo{g}", name=f"wo{g}")
                nc.sync.dma_start(t[:], wo_d[g * 128:(g + 1) * 128, :])
                wo_t.append(t)

            # ---- PE warm-up: dummy matmuls on a memset tile (no DMA dep,
            # starts immediately) while the first slab lands ----
            wjunk = work.tile([128, 256], BF16, tag="wjunk", name="warm_junk")
            nc.vector.memset(wjunk[:], 0.5)
            wps = psum.tile([128, 512], F32, tag="op", bufs=1, name="warm_ps")
            NWARM = 22
            for w in range(NWARM):
                nc.tensor.matmul(wps[:, 0:256], wjunk[:, 0:128], wjunk[:],
                                 start=(w == 0), stop=(w == NWARM - 1))
            wsb = work.tile([128, 16], F32, tag="wsb", name="warm_sb")
            nc.vector.tensor_copy(wsb[:], wps[:, 0:16])

            def keepalive(tag, n):
                # dummy warm matmuls to hold HAM at 8/8 through a PE-idle
                # latency chain (uses a free sc-pool bank, tiny drain)
                kps = psum.tile([128, 1024], F32, tag="sc", bufs=2,
                                name=f"ka_{tag}")
                for w in range(n):
                    nc.tensor.matmul(kps[:, 0:256], wjunk[:, 0:128],
                                     wjunk[:], start=(w == 0),
                                     stop=(w == n - 1))
                ksb = work.tile([128, 16], F32, tag="wsb", name=f"kasb_{tag}")
                nc.vector.tensor_copy(ksb[:], kps[:, 0:16])

            # ---- phase A: v projection (natural layout) ----
            # v_t[l]: [128 tokens, 256 chans] = h0|h1|h2|h3 64 each.
            v_t = [vpool.tile([128, 256], BF16, tag=f"v{l}", name=f"v{l}")
                   for l in range(NL)]

            def phase_a_l(l):
                ps = psum.tile([128, 1024], F32, tag="sc", bufs=2,
                               name=f"psv{l}")
                for k in range(NKC):
                    nc.tensor.matmul(
                        ps[:, 0:256], xt_t[k][:, bass.ts(l, 128)], wv_t[k][:],
                        start=(k == 0), stop=False)
                nc.tensor.matmul(ps[:, 0:256], ones_t[:, 0:128], bv_t[:],
                                 start=False, stop=True)
                nc.scalar.activation(v_t[l][:], ps[:, 0:256], AF.Copy)

            # ---- phase B: q/k projection (transposed) + bias + RoPE ----
            # qkrot[m]: [128 chans, L]; m 0,1 = q (heads 01 | 23), 2,3 = k.
            qkrot = []
            for m in range(4):
                t = qkpool.tile([128, L], BF16, tag=f"qkr{m}", name=f"qkr{m}")
                qkrot.append(t)

            def phase_b_mn(m, n, drain):
                sl = bass.ts(n, 512)
                qkb = work.tile([128, 512], BF16, tag="qkb", bufs=3,
                                name=f"qkb{m}_{n}")
                qsh = work.tile([128, 512], BF16, tag="qsh", bufs=3,
                                name=f"qsh{m}_{n}")
                ps = psum.tile([128, 512], F32, tag="op", bufs=1,
                               name=f"psqk{m}_{n}")
                for k in range(NKC):
                    nc.tensor.matmul(
                        ps[:], wqk_t[k][:, bass.ts(m, 128)],
                        xt_t[k][:, sl],
                        start=(k == 0), stop=(k == NKC - 1))
                if drain == "act":
                    nc.scalar.activation(qkb[:], ps[:], AF.Identity,
                                         bias=bqk_t[:, m:m + 1], scale=1.0)
                else:
                    nc.vector.tensor_scalar(qkb[:], ps[:], bqk_t[:, m:m + 1],
                                            None, op0=ALU.add)
                # rot = qkb*cos + pairswap(qkb)*sin_signed
                nc.vector.stream_shuffle(qsh[:], qkb[:], mask=PAIR_SWAP)
                nc.vector.tensor_mul(qkrot[m][:, sl], qkb[:], cos_t[:, sl])
                nc.vector.tensor_mul(qkb[:], qsh[:], sin_t[:, sl])
                nc.vector.tensor_add(qkrot[m][:, sl],
                                     qkrot[m][:, sl], qkb[:])

            # ---- phase C: attention ----
            # o_ps[g]: [128 = head2g ch | head2g+1 ch, 512 q] per i-slot.
            # S_ps rows 0/32/64/96 = 2*sum(p) per head (col 64*? no: via the
            # duplicated 2.0? -- here onesc gives S; factor 2 folded into
            # the reciprocal layout instead: rec = 1/(2S) computed from S).
            oT = [qkpool.tile([128, L], BF16, tag=f"oT{g}", name=f"oT{g}")
                  for g in range(2)]

            pts = {}      # (g, j) -> pt tile for current slot
            o_ps = [None, None]
            s_ps = [None]

            def att_j(i, j):
                nj = 4 * i + 4
                d = j - 4 * i
                vf = max(0, d) * 128
                for g in range(2):
                    q_t, k_t = qkrot[g], qkrot[2 + g]
                    ss = psum.tile([128, 1024], F32, tag="sc", bufs=2,
                                   name=f"ss{g}_{i}_{j}")
                    for h in range(2):
                        nc.tensor.matmul(
                            ss[:, h * 512 + vf:(h + 1) * 512],
                            k_t[h * 64:h * 64 + 64, bass.ts(j, 128)],
                            q_t[h * 64:h * 64 + 64,
                                i * 512 + vf:(i + 1) * 512],
                            start=True, stop=True, skip_group_check=True)
                    pt = ptpool.tile([128, 1024], BF16, tag="pt",
                                     name=f"pt{g}_{i}_{j}")
                    nc.scalar.activation(
                        pt[:].rearrange("p (h q) -> p h q", h=2)[:, :, vf:512],
                        ss[:].rearrange("p (h q) -> p h q", h=2)[:, :, vf:512],
                        AF.Exp, scale=0.125)
                    if d >= 0:
                        nc.gpsimd.affine_select(
                            pt[:].rearrange(
                                "p (h q) -> p h q", h=2)[:, :, vf:vf + 128],
                            pt[:].rearrange(
                                "p (h q) -> p h q", h=2)[:, :, vf:vf + 128],
                            pattern=[[0, 2], [1, 128]],
                            compare_op=ALU.is_ge, fill=0.0,
                            base=0, channel_multiplier=-1)
                    pts[(g, j)] = pt

            def av_unit(i, jj):
                nj = 4 * i + 4
                vf = max(0, jj - 4 * i) * 128
                if jj == 0:
                    o_ps[0] = psum.tile([128, 512], F32, tag="oacc", bufs=2,
                                        name=f"oacc0_{i}")
                    o_ps[1] = psum.tile([128, 512], F32, tag="oacc", bufs=2,
                                        name=f"oacc1_{i}")
                    s_ps[0] = psum.tile([128, 512], F32, tag="spsum", bufs=1,
                                        name=f"sps{i}")
                for g in range(2):
                    pt = pts[(g, jj)]
                    for h in range(2):
                        nc.tensor.matmul(
                            o_ps[g][64 * h:64 * h + 64, vf:512],
                            v_t[jj][:, 128 * g + 64 * h:128 * g + 64 * h + 64],
                            pt[:, 512 * h + vf:512 * h + 512],
                            start=(jj == 0), stop=(jj == nj - 1),
                            skip_group_check=True)
                for r in range(4):
                    g, h = r // 2, r % 2
                    pt = pts[(g, jj)]
                    nc.tensor.matmul(
                        s_ps[0][32 * r:32 * r + 1, vf:512],
                        onesc_t[:],
                        pt[:, 512 * h + vf:512 * h + 512],
                        start=(jj == 0), stop=(jj == nj - 1),
                        skip_group_check=True, tile_position=(0, 32 * r))
                for g in range(2):
                    del pts[(g, jj)]

            # swish state per slot i
            sw_state = {}

            def swish_a(i, tail=False):
                # drains + reciprocal chain; 1/(2S) rows land in rec1.
                # S drains FIRST (on ACT, which reads PSUM fast and is idle
                # here) -- it heads the DMA->recip->DMA critical path; the o
                # drains follow on DVE in parallel.
                s_sb = swpool.tile([97, 512], F32, tag="ssb", name=f"ssb{i}")
                nc.scalar.activation(s_sb[:], s_ps[0][0:97, :], AF.Copy)
                o_sb = [None, None]
                for g in range(2):
                    o_sb[g] = swpool.tile([128, 512], BF16, tag=f"osb{g}",
                                          name=f"osb{g}_{i}")
                    nc.vector.tensor_copy(o_sb[g][:], o_ps[g][:])
                st = swpool.tile([128, 16], F32, tag="st", name=f"st{i}")
                # tail: use the ACT hwdge ring (sync ring is flushing output
                # blocks); mid-kernel: sync ring (an ACT-ring DMA that waits
                # on the DVE recip would stall the ACT FIFO ahead of the next
                # slot's exps)
                dge = nc.scalar if tail else nc.sync
                dge.dma_start(st[:], s_sb[0:97:32, :])
                # rect = 1/(2S): fold the 2 into the reciprocal input scale
                st2 = swpool.tile([128, 16], F32, tag="st2", name=f"st2{i}")
                nc.vector.tensor_scalar(st2[:], st[:], 2.0, None, op0=ALU.mult)
                rectf = swpool.tile([128, 16], F32, tag="rectf",
                                    name=f"rectf{i}")
                nc.vector.reciprocal(rectf[:], st2[:])
                rect = swpool.tile([128, 16], BF16, tag="rect",
                                   name=f"rect{i}")
                nc.vector.tensor_copy(rect[:], rectf[:])
                rec1 = swpool.tile([1, 2048], BF16, tag="rec1",
                                   name=f"rec1{i}")
                dge.dma_start(rec1[:], rect[:])
                if debug_taps:
                    nc.sync.dma_start(taps["ssb"][i], s_sb[:])
                    nc.sync.dma_start(taps["rec1"][i], rec1[:])
                    nc.sync.dma_start(taps["osb"][i, 0], o_sb[0][:])
                    nc.sync.dma_start(taps["osb"][i, 1], o_sb[1][:])
                sw_state[i] = (o_sb, rec1)

            def swish_b(i, g):
                o_sb, rec1 = sw_state[i]
                t = swpool.tile([128, 512], BF16, tag=f"t{g}",
                                name=f"t{g}_{i}")
                bcp = psum.tile([128, 512], F32, tag="op", bufs=1,
                                name=f"bcp{g}_{i}")
                for h in range(2):
                    r = 2 * g + h
                    nc.tensor.matmul(
                        bcp[64 * h:64 * h + 64, :],
                        ones_t[0:1, 0:64],
                        rec1[0:1, 512 * r:512 * r + 512],
                        start=True, stop=(h == 1), skip_group_check=True)
                nc.vector.tensor_mul(t[:], o_sb[g][:], bcp[:])
                th = swpool.tile([128, 512], BF16, tag=f"th{g}",
                                 name=f"th{g}_{i}")
                nc.scalar.activation(th[:], t[:], AF.Tanh)
                nc.vector.scalar_tensor_tensor(
                    oT[g][:, bass.ts(i, 512)],
                    th[:], 1.0, t[:], op0=ALU.add, op1=ALU.mult)

            def oproj_block(l, tail=False):
                ost = swpool.tile([128, 1024], BF16, tag="ost", bufs=3,
                                  name=f"ost{l}")
                if tail:
                    ps = psum.tile([128, 1024], F32, tag="sc", bufs=2,
                                   name=f"pso{l}")
                    for n in range(2):
                        for g in range(2):
                            nc.tensor.matmul(
                                ps[:, bass.ts(n, 512)],
                                oT[g][:, bass.ts(l, 128)],
                                wo_t[g][:, bass.ts(n, 512)],
                                start=(g == 0), stop=(g == 1))
                    nc.vector.tensor_copy(ost[:, 0:512], ps[:, 0:512])
                    nc.scalar.activation(ost[:, 512:1024], ps[:, 512:1024],
                                         AF.Copy)
                    nc.scalar.dma_start(out_d[bass.ts(l, 128), :], ost[:])
                    return
                else:
                    for n in range(2):
                        ps = psum.tile([128, 512], F32, tag="op", bufs=1,
                                       name=f"pso{l}_{n}")
                        for g in range(2):
                            nc.tensor.matmul(
                                ps[:], oT[g][:, bass.ts(l, 128)],
                                wo_t[g][:, bass.ts(n, 512)],
                                start=(g == 0), stop=(g == 1))
                        nc.vector.tensor_copy(ost[:, bass.ts(n, 512)], ps[:])
                nc.sync.dma_start(out_d[bass.ts(l, 128), :], ost[:])

            # ---- emission schedule ----
            for s in range(NLQ):
                for l in range(4 * s, 4 * s + 4):
                    phase_a_l(l)
                phase_b_mn(0, s, drain="act")
                phase_b_mn(2, s, drain="act")
                if s == 1:
                    phase_b_mn(1, 0, drain="dve")
                    phase_b_mn(3, 0, drain="dve")
                if s == 2:
                    att_j(0, 0)
                    att_j(0, 1)
                if s == 3:
                    att_j(0, 2)
                    av_unit(0, 0)
                    att_j(0, 3)
                    av_unit(0, 1)
                    phase_b_mn(1, 1, drain="dve")
                    av_unit(0, 2)
                    phase_b_mn(3, 1, drain="dve")
                    av_unit(0, 3)
                    swish_a(0)

            for i in range(1, NLQ):
                nj = 4 * i + 4
                # oproj l-blocks of slot i-1 spliced into the j-loop;
                # deferred swish finals of slot i-1 at j=0/1; pair-1
                # projection chunks for the NEXT slot at j=2/3.
                opl = list(range(4 * (i - 1), 4 * (i - 1) + 4))
                step = max(1, (nj - 4) // 4)
                splice = {4 + step * k: opl[k] for k in range(4)}
                assert max(splice) < nj
                for j in range(nj):
                    att_j(i, j)
                    if j >= LAG:
                        av_unit(i, j - LAG)
                    if j == 0:
                        swish_b(i - 1, 0)
                    if j == 1:
                        swish_b(i - 1, 1)
                    if j == 2 and i + 1 < NLQ:
                        phase_b_mn(1, i + 1, drain="dve")
                    if j == 3 and i + 1 < NLQ:
                        phase_b_mn(3, i + 1, drain="dve")
                    if j in splice:
                        oproj_block(splice[j])
                for jj in range(nj - LAG, nj):
                    av_unit(i, jj)
                if i == NLQ - 1:
                    keepalive("tail", 36)
                swish_a(i, tail=(i == NLQ - 1))

            swish_b(NLQ - 1, 0)
            swish_b(NLQ - 1, 1)
            for l in range(4 * (NLQ - 1), 4 * NLQ):
                oproj_block(l, tail=True)
            if debug_taps:
                for m in range(4):
                    nc.sync.dma_start(taps["qkr"][m], qkrot[m][:])
                for l in range(NL):
                    nc.sync.dma_start(taps["v"][l], v_t[l][:])
                for g in range(2):
                    nc.sync.dma_start(taps["oT"][g], oT[g][:])

    nc.compile()
    return nc


def _rope_tables():
    f2 = 1.0 / (10000.0 ** (np.arange(0, HD, 4, dtype=np.float64)[:HD // 4] / HD))
    s = np.arange(LS, dtype=np.float64)
    ang_s = np.zeros((LS, HD // 2), dtype=np.float64)
    ang_s[:, :HD // 4] = np.outer(s % L1D, f2)
    ang_s[:, HD // 4:] = np.outer(s // L1D, f2)
    f1 = 1.0 / (10000.0 ** (np.arange(0, HD, 2, dtype=np.float64) / HD))
    ang_t = np.outer(np.arange(NT, dtype=np.float64), f1)
    l = np.arange(L)
    ang = ang_s[l % LS] + ang_t[l // LS]        # [L, 32]
    pair = (np.arange(128) % HD) // 2           # [128] -> pair index
    cosrep = np.cos(ang).T[pair].astype(np.float32)  # [128, L]
    sinrep = np.sin(ang).T[pair].astype(np.float32)
    # fold the pair-swap signs into sin: row 2i gets -sin (it receives
    # -q[2i+1]*sin), row 2i+1 gets +sin.
    sinrep[0::2, :] *= -1.0
    return np.ascontiguousarray(cosrep), np.ascontiguousarray(sinrep)


def _make_in_maps(inp):
    x = np.asarray(inp["x"], dtype=np.float32)
    w_qkv = np.asarray(inp["w_qkv"], dtype=np.float32)
    b_qkv = np.asarray(inp["b_qkv"], dtype=np.float32)
    w_out = np.asarray(inp["w_out"], dtype=np.float32)
    if "consts" not in _CACHE:
        _CACHE["consts"] = _rope_tables()
    cosrep, sinsig = _CACHE["consts"]
    in_maps = []
    for c in range(N_CORES):
        b = c // 4
        heads = [4 * (c % 4) + i for i in range(HPC)]
        qrows = [h * 192 + j for h in heads for j in range(64)]
        krows = [h * 192 + 64 + j for h in heads for j in range(64)]
        vrows = [h * 192 + 128 + j for h in heads for j in range(64)]
        ocols = [h * 64 + j for h in heads for j in range(64)]
        bf = ml_dtypes.bfloat16
        wqk = np.ascontiguousarray(w_qkv[qrows + krows, :].T).astype(bf)
        bqkc = np.ascontiguousarray(
            b_qkv[qrows + krows].reshape(4, 128).T).astype(np.float32)
        wv = np.ascontiguousarray(w_qkv[vrows, :].T).astype(bf)
        bv = np.ascontiguousarray(b_qkv[vrows].reshape(1, 256)).astype(bf)
        woT = np.ascontiguousarray(w_out[:, ocols].T).astype(bf)
        xt = np.ascontiguousarray(x[b].T).astype(bf)
        in_maps.append({
            "xt": xt, "wqk": wqk, "bqkc": bqkc, "wv": wv, "bv": bv,
            "cosrep": cosrep.astype(bf), "sinsig": sinsig.astype(bf),
            "woT": woT,
        })
    return in_maps


def kernel(x, w_qkv, b_qkv, w_out, b_out):
    b_out = np.asarray(b_out, dtype=np.float32)
    if "nc" not in _CACHE:
        _CACHE["nc"] = _build()
    nc = _CACHE["nc"]
    in_maps = _make_in_maps({"x": x, "w_qkv": w_qkv, "b_qkv": b_qkv,
                             "w_out": w_out})

    res = run_bass_kernel_spmd(nc, in_maps, core_ids=list(range(N_CORES)))

    out = np.zeros((B, L, H), dtype=np.float32)
    for c in range(N_CORES):
        out[c // 4] += res.results[c]["out"].astype(np.float32)
    out += b_out[None, None, :]
    return out


# revision 15
# speedup vs baseline: 1.2712x; 1.2712x over previous
"""Trainium2 Bass kernel for nn_MultiHeadAttention_79465484911033.

Sharding: 8 cores = 2 batches x 4 head-groups (4 heads each of 16).
Each core: QKV projection for its heads (column-parallel), RoPE
(spatial+temporal angles composed into one rotation), causal attention,
swish, and a row-parallel partial output projection. Host sums the 4
partials per batch and adds b_out.

v2 design notes (vs the 187.7us baseline):
- RoPE shuffle via DVE stream_shuffle (32-lane pair-swap permute) with
  the sign folded into the sin table; kills the PE shuffle matmuls and
  one psum drain per chunk.
- q/k bias added during the psum drain (ACT Identity+bias-vector for
  slab-phase chunks, DVE tensor_scalar for attention-phase chunks);
  kills the bias matmuls.
- Scores diagonal blocks compute only the causal q-range (N=512-128d).
- attn@v: both heads of a pair run as one col-tiled concurrent MM pair
  (M=64 at col-groups 0/64) on full [128,128] v stationary (FWL-able);
  the softmax denominators for all 4 heads accumulate via 4 concurrent
  col-tiled M=1 ones-matmuls into a separate psum bank. 3 psum-stream
  slots per k-block instead of 4.
- swish: o/(2S) with S reshaped [128,16] via DMA for the cheap DVE
  reciprocal, broadcast via gpsimd mid-kernel but via PE K=1 matmuls
  for the last slot (keeps PE warm through the tail, avoids the ~2us
  gpsimd wake+drain latency before the final oproj).
- attention phase is ACT(exp)-bound; PE filled with pair-1 projection
  chunks and oproj blocks spliced between j-steps.
- oproj drains on DVE mid-kernel, DVE+ACT alternating for the tail
  block which uses the (then-free) score psum banks.
"""

import sys

for _p in ("/opt/trn_rl_repo", "/root/.axon_site/_ro/trn_rl_repo"):
    if _p not in sys.path:
        sys.path.append(_p)

import numpy as np
import ml_dtypes

import concourse.bass as bass
import concourse.mybir as mybir
import concourse.tile as tile
from concourse import bacc
from concourse.bass_utils import run_bass_kernel_spmd

F32 = mybir.dt.float32
BF16 = mybir.dt.bfloat16
FP8 = mybir.dt.float8e4
AF = mybir.ActivationFunctionType
ALU = mybir.AluOpType

B, L, H = 2, 2048, 1024
SX8, SW8 = 32.0, 512.0          # fp8 pre-scales for x and w_qk
DESC8 = 1.0 / (SX8 * SW8)       # folded into the q/k psum drain
NH, HD = 16, 64
NT, LS, L1D = 8, 256, 16
N_CORES = 8
HPC = 4               # heads per core
NKC = H // 128        # 8 contraction chunks
NL = L // 128         # 16 L chunks of 128
NLQ = L // 512        # 4 L tiles of 512
LAG = 2

_CACHE = {}

PAIR_SWAP = []
for _i in range(16):
    PAIR_SWAP += [2 * _i + 1, 2 * _i]


def _build(debug_taps=False):
    nc = bacc.Bacc("TRN2", target_bir_lowering=False, debug=False,
                   enable_asserts=True, num_devices=N_CORES)
    taps = {}
    if debug_taps:
        taps["qkr"] = nc.dram_tensor("dbg_qkr", [4, 128, L], BF16,
                                     kind="ExternalOutput")
        taps["v"] = nc.dram_tensor("dbg_v", [NL, 128, 256], BF16,
                                   kind="ExternalOutput")
        taps["oT"] = nc.dram_tensor("dbg_oT", [2, 128, L], BF16,
                                    kind="ExternalOutput")
        taps["ssb"] = nc.dram_tensor("dbg_ssb", [NLQ, 97, 512], F32,
                                     kind="ExternalOutput")
        taps["rec1"] = nc.dram_tensor("dbg_rec1", [NLQ, 1, 2048], BF16,
                                      kind="ExternalOutput")
        taps["osb"] = nc.dram_tensor("dbg_osb", [NLQ, 2, 128, 512], BF16,
                                     kind="ExternalOutput")

    xt_d = nc.dram_tensor("xt", [H, L], BF16, kind="ExternalInput")
    xt8_d = nc.dram_tensor("xt8", [512, 2 * L], FP8, kind="ExternalInput")
    wqk_d = nc.dram_tensor("wqk8", [512, 1024], FP8, kind="ExternalInput")
    bqk_d = nc.dram_tensor("bqkc", [128, 4], F32, kind="ExternalInput")
    wv_d = nc.dram_tensor("wv", [H, 256], BF16, kind="ExternalInput")
    bv_d = nc.dram_tensor("bv", [1, 256], BF16, kind="ExternalInput")
    cos_d = nc.dram_tensor("cosrep", [128, L], BF16, kind="ExternalInput")
    sin_d = nc.dram_tensor("sinsig", [128, L], BF16, kind="ExternalInput")
    wo_d = nc.dram_tensor("woT", [256, 1024], BF16, kind="ExternalInput")
    out_d = nc.dram_tensor("out", [L, H], BF16, kind="ExternalOutput")

    NK8 = 4                      # 256-deep fp8 contraction chunks
    with tile.TileContext(nc) as tc:
        with (
            tc.tile_pool(name="const", bufs=1) as cpool,
            tc.tile_pool(name="xt", bufs=1) as xpool,
            tc.tile_pool(name="w", bufs=1) as wpool,
            tc.tile_pool(name="qk", bufs=1) as qkpool,
            tc.tile_pool(name="v", bufs=1) as vpool,
            tc.tile_pool(name="work", bufs=1) as work,
            tc.tile_pool(name="pt", bufs=10) as ptpool,
            tc.tile_pool(name="sw", bufs=2) as swpool,
            tc.tile_pool(name="ps", bufs=1, space="PSUM") as psum,
        ):
            cos_t = cpool.tile([128, L], BF16, tag="cos")
            sin_t = cpool.tile([128, L], BF16, tag="sin")
            bqk_t = cpool.tile([128, 4], F32, tag="bqk")
            bv_t = cpool.tile([1, 256], BF16, tag="bv")
            ones_t = cpool.tile([1, 512], BF16, tag="ones")
            onesc_t = cpool.tile([128, 1], BF16, tag="onesc")
            nc.vector.memset(ones_t[:], 1.0)
            nc.vector.memset(onesc_t[:], 1.0)

            # prime the exp/tanh activation table set during initial DMA
            prm = cpool.tile([1, 8], F32, tag="prm")
            prm2 = cpool.tile([1, 8], F32, tag="prm2")
            nc.vector.memset(prm[:], 0.0)
            nc.scalar.activation(prm2[:], prm[:], AF.Exp)

            wv_t = []
            for k in range(NKC):
                t = wpool.tile([128, 256], BF16, tag=f"wv{k}", name=f"wv{k}")
                nc.scalar.dma_start(t[:], wv_d[k * 128:(k + 1) * 128, :])
                wv_t.append(t)
            nc.scalar.dma_start(bv_t[:], bv_d[:])
            xt_t = [xpool.tile([128, L], BF16, tag=f"xt{k}", name=f"xt{k}")
                    for k in range(NKC)]
            xt8_t = [xpool.tile([128, 2 * L], FP8, tag=f"xt8_{k}",
                                name=f"xt8_{k}") for k in range(NK8)]
            wqk_t = [wpool.tile([128, 1024], FP8, tag=f"wqk{k}",
                                name=f"wqk{k}") for k in range(NK8)]

            def dma_slab(s):
                sl = bass.ts(s, 512)
                for k in range(NKC):
                    nc.sync.dma_start(xt_t[k][:, sl],
                                      xt_d[k * 128:(k + 1) * 128, sl])
                for k in range(NK8):
                    nc.sync.dma_start(
                        xt8_t[k][:].rearrange("p (w n) -> p w n",
                                              w=2)[:, :, sl],
                        xt8_d[k * 128:(k + 1) * 128, :].rearrange(
                            "r (w n) -> r w n", w=2)[:, :, sl])
                nc.sync.dma_start(cos_t[:, sl], cos_d[:, sl])
                nc.sync.dma_start(sin_t[:, sl], sin_d[:, sl])

            for k in range(NK8):
                nc.scalar.dma_start(wqk_t[k][:],
                                    wqk_d[k * 128:(k + 1) * 128, :])
            nc.scalar.dma_start(bqk_t[:], bqk_d[:])
            dma_slab(0)
            for s in range(1, NLQ):
                dma_slab(s)
            wo_t = []
            for g in range(2):
                t = wpool.tile([128, 1024], BF16, tag=f"wo{g}", name=f"wo{g}")
                nc.scalar.dma_start(t[:], wo_d[g * 128:(g + 1) * 128, :])
                wo_t.append(t)

            # ---- PE warm-up: dummy matmuls on a memset tile (no DMA dep,
            # starts immediately) while the first slab lands ----
            wjunk = work.tile([128, 256], BF16, tag="wjunk", name="warm_junk")
            nc.vector.memset(wjunk[:], 0.5)
            wps = psum.tile([128, 512], F32, tag="op", bufs=1, name="warm_ps")
            NWARM = 22
            for w in range(NWARM):
                nc.tensor.matmul(wps[:, 0:256], wjunk[:, 0:128], wjunk[:],
                                 start=(w == 0), stop=(w == NWARM - 1))
            wsb = work.tile([128, 16], F32, tag="wsb", name="warm_sb")
            nc.vector.tensor_copy(wsb[:], wps[:, 0:16])

            def keepalive(tag, n):
                # dummy warm matmuls to hold HAM at 8/8 through a PE-idle
                # latency chain (uses a free sc-pool bank, tiny drain)
                kps = psum.tile([128, 1024], F32, tag="sc", bufs=2,
                                name=f"ka_{tag}")
                for w in range(n):
                    nc.tensor.matmul(kps[:, 0:256], wjunk[:, 0:128],
                                     wjunk[:], start=(w == 0),
                                     stop=(w == n - 1))
                ksb = work.tile([128, 16], F32, tag="wsb", name=f"kasb_{tag}")
                nc.vector.tensor_copy(ksb[:], kps[:, 0:16])

            # ---- phase A: v projection (natural layout) ----
            # v_t[l]: [128 tokens, 256 chans] = h0|h1|h2|h3 64 each.
            v_t = [vpool.tile([128, 256], BF16, tag=f"v{l}", name=f"v{l}")
                   for l in range(NL)]

            def phase_a_l(l):
                ps = psum.tile([128, 1024], F32, tag="sc", bufs=2,
                               name=f"psv{l}")
                for k in range(NKC):
                    nc.tensor.matmul(
                        ps[:, 0:256], xt_t[k][:, bass.ts(l, 128)], wv_t[k][:],
                        start=(k == 0), stop=False)
                nc.tensor.matmul(ps[:, 0:256], ones_t[:, 0:128], bv_t[:],
                                 start=False, stop=True)
                nc.scalar.activation(v_t[l][:], ps[:, 0:256], AF.Copy)

            # ---- phase B: q/k projection (transposed) + bias + RoPE ----
            # qkrot[m]: [128 chans, L]; m 0,1 = q (heads 01 | 23), 2,3 = k.
            qkrot = []
            for m in range(4):
                t = qkpool.tile([128, L], BF16, tag=f"qkr{m}", name=f"qkr{m}")
                qkrot.append(t)

            def phase_b_mn(m, n, drain):
                sl = bass.ts(n, 512)
                qkb = work.tile([128, 512], BF16, tag="qkb", bufs=3,
                                name=f"qkb{m}_{n}")
                qsh = work.tile([128, 512], BF16, tag="qsh", bufs=3,
                                name=f"qsh{m}_{n}")
                ps = psum.tile([128, 512], F32, tag="op", bufs=1,
                               name=f"psqk{m}_{n}")
                for k in range(NK8):
                    nc.tensor.matmul(
                        ps[:],
                        wqk_t[k][:].rearrange("p (w m) -> p w m",
                                              w=2)[:, :, bass.ts(m, 128)],
                        xt8_t[k][:].rearrange("p (w n) -> p w n",
                                              w=2)[:, :, sl],
                        start=(k == 0), stop=(k == NK8 - 1),
                        perf_mode=mybir.MatmulPerfMode.DoubleRow)
                if drain == "act":
                    nc.scalar.activation(qkb[:], ps[:], AF.Identity,
                                         bias=bqk_t[:, m:m + 1], scale=DESC8)
                else:
                    nc.vector.tensor_scalar(qkb[:], ps[:], DESC8,
                                            bqk_t[:, m:m + 1],
                                            op0=ALU.mult, op1=ALU.add)
                # rot = qkb*cos + pairswap(qkb)*sin_signed
                nc.vector.stream_shuffle(qsh[:], qkb[:], mask=PAIR_SWAP)
                nc.vector.tensor_mul(qkrot[m][:, sl], qkb[:], cos_t[:, sl])
                nc.vector.tensor_mul(qkb[:], qsh[:], sin_t[:, sl])
                nc.vector.tensor_add(qkrot[m][:, sl],
                                     qkrot[m][:, sl], qkb[:])

            # ---- phase C: attention ----
            # o_ps[g]: [128 = head2g ch | head2g+1 ch, 512 q] per i-slot.
            # S_ps rows 0/32/64/96 = 2*sum(p) per head (col 64*? no: via the
            # duplicated 2.0? -- here onesc gives S; factor 2 folded into
            # the reciprocal layout instead: rec = 1/(2S) computed from S).
            oT = [qkpool.tile([128, L], BF16, tag=f"oT{g}", name=f"oT{g}")
                  for g in range(2)]

            pts = {}      # (g, j) -> pt tile for current slot
            o_ps = [None, None]
            s_ps = [None]

            def att_j(i, j):
                nj = 4 * i + 4
                d = j - 4 * i
                vf = max(0, d) * 128
                for g in range(2):
                    q_t, k_t = qkrot[g], qkrot[2 + g]
                    ss = psum.tile([128, 1024], F32, tag="sc", bufs=2,
                                   name=f"ss{g}_{i}_{j}")
                    for h in range(2):
                        nc.tensor.matmul(
                            ss[:, h * 512 + vf:(h + 1) * 512],
                            k_t[h * 64:h * 64 + 64, bass.ts(j, 128)],
                            q_t[h * 64:h * 64 + 64,
                                i * 512 + vf:(i + 1) * 512],
                            start=True, stop=True, skip_group_check=True)
                    pt = ptpool.tile([128, 1024], BF16, tag="pt",
                                     name=f"pt{g}_{i}_{j}")
                    nc.scalar.activation(
                        pt[:].rearrange("p (h q) -> p h q", h=2)[:, :, vf:512],
                        ss[:].rearrange("p (h q) -> p h q", h=2)[:, :, vf:512],
                        AF.Exp, scale=0.125)
                    if d >= 0:
                        nc.gpsimd.affine_select(
                            pt[:].rearrange(
                                "p (h q) -> p h q", h=2)[:, :, vf:vf + 128],
                            pt[:].rearrange(
                                "p (h q) -> p h q", h=2)[:, :, vf:vf + 128],
                            pattern=[[0, 2], [1, 128]],
                            compare_op=ALU.is_ge, fill=0.0,
                            base=0, channel_multiplier=-1)
                    pts[(g, j)] = pt

            def av_unit(i, jj):
                nj = 4 * i + 4
                vf = max(0, jj - 4 * i) * 128
                if jj == 0:
                    o_ps[0] = psum.tile([128, 512], F32, tag="oacc", bufs=2,
                                        name=f"oacc0_{i}")
                    o_ps[1] = psum.tile([128, 512], F32, tag="oacc", bufs=2,
                                        name=f"oacc1_{i}")
                    s_ps[0] = psum.tile([128, 512], F32, tag="spsum", bufs=1,
                                        name=f"sps{i}")
                for g in range(2):
                    pt = pts[(g, jj)]
                    for h in range(2):
                        nc.tensor.matmul(
                            o_ps[g][64 * h:64 * h + 64, vf:512],
                            v_t[jj][:, 128 * g + 64 * h:128 * g + 64 * h + 64],
                            pt[:, 512 * h + vf:512 * h + 512],
                            start=(jj == 0), stop=(jj == nj - 1),
                            skip_group_check=True)
                for r in range(4):
                    g, h = r // 2, r % 2
                    pt = pts[(g, jj)]
                    nc.tensor.matmul(
                        s_ps[0][32 * r:32 * r + 1, vf:512],
                        onesc_t[:],
                        pt[:, 512 * h + vf:512 * h + 512],
                        start=(jj == 0), stop=(jj == nj - 1),
                        skip_group_check=True, tile_position=(0, 32 * r))
                for g in range(2):
                    del pts[(g, jj)]

            # swish state per slot i
            sw_state = {}

            def swish_a(i, tail=False):
                # drains + reciprocal chain; 1/(2S) rows land in rec1.
                # S drains FIRST (on ACT, which reads PSUM fast and is idle
                # here) -- it heads the DMA->recip->DMA critical path; the o
                # drains follow on DVE in parallel.
                s_sb = swpool.tile([97, 512], F32, tag="ssb", name=f"ssb{i}")
                nc.scalar.activation(s_sb[:], s_ps[0][0:97, :], AF.Copy)
                o_sb = [None, None]
                for g in range(2):
                    o_sb[g] = swpool.tile([128, 512], BF16, tag=f"osb{g}",
                                          name=f"osb{g}_{i}")
                    nc.vector.tensor_copy(o_sb[g][:], o_ps[g][:])
                st = swpool.tile([128, 16], F32, tag="st", name=f"st{i}")
                # tail: use the ACT hwdge ring (sync ring is flushing output
                # blocks); mid-kernel: sync ring (an ACT-ring DMA that waits
                # on the DVE recip would stall the ACT FIFO ahead of the next
                # slot's exps)
                dge = nc.scalar if tail else nc.sync
                dge.dma_start(st[:], s_sb[0:97:32, :])
                # rect = 1/(2S): fold the 2 into the reciprocal input scale
                st2 = swpool.tile([128, 16], F32, tag="st2", name=f"st2{i}")
                nc.vector.tensor_scalar(st2[:], st[:], 2.0, None, op0=ALU.mult)
                rectf = swpool.tile([128, 16], F32, tag="rectf",
                                    name=f"rectf{i}")
                nc.vector.reciprocal(rectf[:], st2[:])
                rect = swpool.tile([128, 16], BF16, tag="rect",
                                   name=f"rect{i}")
                nc.vector.tensor_copy(rect[:], rectf[:])
                rec1 = swpool.tile([1, 2048], BF16, tag="rec1",
                                   name=f"rec1{i}")
                dge.dma_start(rec1[:], rect[:])
                if debug_taps:
                    nc.sync.dma_start(taps["ssb"][i], s_sb[:])
                    nc.sync.dma_start(taps["rec1"][i], rec1[:])
                    nc.sync.dma_start(taps["osb"][i, 0], o_sb[0][:])
                    nc.sync.dma_start(taps["osb"][i, 1], o_sb[1][:])
                sw_state[i] = (o_sb, rec1)

            def swish_b(i, g):
                o_sb, rec1 = sw_state[i]
                t = swpool.tile([128, 512], BF16, tag=f"t{g}",
                                name=f"t{g}_{i}")
                bcp = psum.tile([128, 512], F32, tag="op", bufs=1,
                                name=f"bcp{g}_{i}")
                for h in range(2):
                    r = 2 * g + h
                    nc.tensor.matmul(
                        bcp[64 * h:64 * h + 64, :],
                        ones_t[0:1, 0:64],
                        rec1[0:1, 512 * r:512 * r + 512],
                        start=True, stop=(h == 1), skip_group_check=True)
                nc.vector.tensor_mul(t[:], o_sb[g][:], bcp[:])
                th = swpool.tile([128, 512], BF16, tag=f"th{g}",
                                 name=f"th{g}_{i}")
                nc.scalar.activation(th[:], t[:], AF.Tanh)
                nc.vector.scalar_tensor_tensor(
                    oT[g][:, bass.ts(i, 512)],
                    th[:], 1.0, t[:], op0=ALU.add, op1=ALU.mult)

            def oproj_block(l, tail=False):
                ost = swpool.tile([128, 1024], BF16, tag="ost", bufs=3,
                                  name=f"ost{l}")
                if tail:
                    ps = psum.tile([128, 1024], F32, tag="sc", bufs=2,
                                   name=f"pso{l}")
                    for n in range(2):
                        for g in range(2):
                            nc.tensor.matmul(
                                ps[:, bass.ts(n, 512)],
                                oT[g][:, bass.ts(l, 128)],
                                wo_t[g][:, bass.ts(n, 512)],
                                start=(g == 0), stop=(g == 1))
                    nc.vector.tensor_copy(ost[:, 0:512], ps[:, 0:512])
                    nc.scalar.activation(ost[:, 512:1024], ps[:, 512:1024],
                                         AF.Copy)
                    nc.scalar.dma_start(out_d[bass.ts(l, 128), :], ost[:])
                    return
                else:
                    for n in range(2):
                        ps = psum.tile([128, 512], F32, tag="op", bufs=1,
                                       name=f"pso{l}_{n}")
                        for g in range(2):
                            nc.tensor.matmul(
                                ps[:], oT[g][:, bass.ts(l, 128)],
                                wo_t[g][:, bass.ts(n, 512)],
                                start=(g == 0), stop=(g == 1))
                        nc.vector.tensor_copy(ost[:, bass.ts(n, 512)], ps[:])
                nc.sync.dma_start(out_d[bass.ts(l, 128), :], ost[:])

            # ---- emission schedule ----
            for s in range(NLQ):
                for l in range(4 * s, 4 * s + 4):
                    phase_a_l(l)
                phase_b_mn(0, s, drain="act")
                phase_b_mn(2, s, drain="act")
                if s == 1:
                    phase_b_mn(1, 0, drain="dve")
                    phase_b_mn(3, 0, drain="dve")
                if s == 2:
                    att_j(0, 0)
                    att_j(0, 1)
                if s == 3:
                    att_j(0, 2)
                    av_unit(0, 0)
                    att_j(0, 3)
                    av_unit(0, 1)
                    phase_b_mn(1, 1, drain="dve")
                    av_unit(0, 2)
                    phase_b_mn(3, 1, drain="dve")
                    av_unit(0, 3)
                    swish_a(0)

            for i in range(1, NLQ):
                nj = 4 * i + 4
                # oproj l-blocks of slot i-1 spliced into the j-loop;
                # deferred swish finals of slot i-1 at j=0/1; pair-1
                # projection chunks for the NEXT slot at j=2/3.
                opl = list(range(4 * (i - 1), 4 * (i - 1) + 4))
                step = max(1, (nj - 4) // 4)
                splice = {4 + step * k: opl[k] for k in range(4)}
                assert max(splice) < nj
                for j in range(nj):
                    att_j(i, j)
                    if j >= LAG:
                        av_unit(i, j - LAG)
                    if j == 0:
                        swish_b(i - 1, 0)
                    if j == 1:
                        swish_b(i - 1, 1)
                    if j == 2 and i + 1 < NLQ:
                        phase_b_mn(1, i + 1, drain="dve")
                    if j == 3 and i + 1 < NLQ:
                        phase_b_mn(3, i + 1, drain="dve")
                    if j in splice:
                        oproj_block(splice[j])
                for jj in range(nj - LAG, nj):
                    av_unit(i, jj)
                if i == NLQ - 1:
                    keepalive("tail", 36)
                swish_a(i, tail=(i == NLQ - 1))

            swish_b(NLQ - 1, 0)
            swish_b(NLQ - 1, 1)
            for l in range(4 * (NLQ - 1), 4 * NLQ):
                oproj_block(l, tail=True)
            if debug_taps:
                for m in range(4):
                    nc.sync.dma_start(taps["qkr"][m], qkrot[m][:])
                for l in range(NL):
                    nc.sync.dma_start(taps["v"][l], v_t[l][:])
                for g in range(2):
                    nc.sync.dma_start(taps["oT"][g], oT[g][:])

    nc.compile()
    return nc


def _rope_tables():
    f2 = 1.0 / (10000.0 ** (np.arange(0, HD, 4, dtype=np.float64)[:HD // 4] / HD))
    s = np.arange(LS, dtype=np.float64)
    ang_s = np.zeros((LS, HD // 2), dtype=np.float64)
    ang_s[:, :HD // 4] = np.outer(s % L1D, f2)
    ang_s[:, HD // 4:] = np.outer(s // L1D, f2)
    f1 = 1.0 / (10000.0 ** (np.arange(0, HD, 2, dtype=np.float64) / HD))
    ang_t = np.outer(np.arange(NT, dtype=np.float64), f1)
    l = np.arange(L)
    ang = ang_s[l % LS] + ang_t[l // LS]        # [L, 32]
    pair = (np.arange(128) % HD) // 2           # [128] -> pair index
    cosrep = np.cos(ang).T[pair].astype(np.float32)  # [128, L]
    sinrep = np.sin(ang).T[pair].astype(np.float32)
    # fold the pair-swap signs into sin: row 2i gets -sin (it receives
    # -q[2i+1]*sin), row 2i+1 gets +sin.
    sinrep[0::2, :] *= -1.0
    return np.ascontiguousarray(cosrep), np.ascontiguousarray(sinrep)


def _make_in_maps(inp):
    x = np.asarray(inp["x"], dtype=np.float32)
    w_qkv = np.asarray(inp["w_qkv"], dtype=np.float32)
    b_qkv = np.asarray(inp["b_qkv"], dtype=np.float32)
    w_out = np.asarray(inp["w_out"], dtype=np.float32)
    if "consts" not in _CACHE:
        _CACHE["consts"] = _rope_tables()
    cosrep, sinsig = _CACHE["consts"]
    in_maps = []
    for c in range(N_CORES):
        b = c // 4
        heads = [4 * (c % 4) + i for i in range(HPC)]
        qrows = [h * 192 + j for h in heads for j in range(64)]
        krows = [h * 192 + 64 + j for h in heads for j in range(64)]
        vrows = [h * 192 + 128 + j for h in heads for j in range(64)]
        ocols = [h * 64 + j for h in heads for j in range(64)]
        bf = ml_dtypes.bfloat16
        f8 = ml_dtypes.float8_e4m3
        wqkf = w_qkv[qrows + krows, :].T * SW8           # [1024 K, 512 M]
        wqk8 = np.ascontiguousarray(
            wqkf.reshape(4, 2, 128, 512).transpose(0, 2, 1, 3)
            .reshape(512, 1024)).astype(f8)
        bqkc = np.ascontiguousarray(
            b_qkv[qrows + krows].reshape(4, 128).T).astype(np.float32)
        wv = np.ascontiguousarray(w_qkv[vrows, :].T).astype(bf)
        bv = np.ascontiguousarray(b_qkv[vrows].reshape(1, 256)).astype(bf)
        woT = np.ascontiguousarray(w_out[:, ocols].T).astype(bf)
        xt = np.ascontiguousarray(x[b].T).astype(bf)
        xt8 = np.ascontiguousarray(
            (x[b].T * SX8).reshape(4, 2, 128, L).transpose(0, 2, 1, 3)
            .reshape(512, 2 * L)).astype(f8)
        in_maps.append({
            "xt": xt, "xt8": xt8, "wqk8": wqk8, "bqkc": bqkc,
            "wv": wv, "bv": bv,
            "cosrep": cosrep.astype(bf), "sinsig": sinsig.astype(bf),
            "woT": woT,
        })
    return in_maps


def kernel(x, w_qkv, b_qkv, w_out, b_out):
    b_out = np.asarray(b_out, dtype=np.float32)
    if "nc" not in _CACHE:
        _CACHE["nc"] = _build()
    nc = _CACHE["nc"]
    in_maps = _make_in_maps({"x": x, "w_qkv": w_qkv, "b_qkv": b_qkv,
                             "w_out": w_out})

    res = run_bass_kernel_spmd(nc, in_maps, core_ids=list(range(N_CORES)))

    out = np.zeros((B, L, H), dtype=np.float32)
    for c in range(N_CORES):
        out[c // 4] += res.results[c]["out"].astype(np.float32)
    out += b_out[None, None, :]
    return out
